# revision 21
# baseline (speedup 1.0000x reference)
"""MultiHeadAttentionBlock (B=2, S=2048, D=1024, H=16, causal) on 8 trn2 cores.

Sharding: tensor-parallel over heads (2 heads / core); an on-device AllToAll
redistributes per-head context so each core computes the output projection for
its 512-row block. Host only slices / casts inputs and concatenates outputs.

fp8 DoubleRow acceleration (0.5 PE cycles/row vs 1.0 for bf16), with
error-compensated operand splits chosen to keep rel err ~1.3e-2 (< 2e-2):
  - Q/K/V projections: weights W' = 32*W split into fp8 hi+lo (host side);
    x_q/x_k quantized to single fp8, x_v compensated (hi+lo, host side).
    Q: 2 terms (Wa xa, Wb xa); K: same + bias via ones-row matmul so the
    eviction can split K into an fp8 (hi, lo) pair; V: 3 terms.
  - Scores: one DR matmul per (j-tile): lhsT slices = (K_hi, K_lo) tile,
    rhs = Q_hi broadcast to both slices -> K side exactly compensated,
    0.5 cycles/row. exp scale 2^-13 absorbs the 32*32 weight scaling.
  - PV + output projection stay bf16 (softmax weights don't survive fp8);
    w_o is pre-divided by 32 host-side to undo the V scale.
Attention per (batch, head-half, i-half) with projection chunks injected as
fillers so the PE stays fed while ScalarE paces the exp stream.
"""

import os
import numpy as np
import ml_dtypes

B, S, D = 2, 2048, 1024
H, DK = 16, 64
ROWS = B * S  # 4096
NCORES = 8
CDIM = 128  # context dims per core (2 heads x 64)
RPC = ROWS // NCORES  # 512 rows per core

BF16 = ml_dtypes.bfloat16
F8 = ml_dtypes.float8_e4m3
WS = 32.0  # weight prescale for fp8
EXPSCALE = 0.125 / (WS * WS)  # 2^-13

_CACHE = {}
LAST_RESULTS = None


def _build_program(with_collective=True):
    import concourse.mybir as mybir
    import concourse.tile as tile
    from concourse import bacc
    from concourse.masks import make_identity

    f32 = mybir.dt.float32
    bf = mybir.dt.bfloat16
    fp8 = mybir.dt.float8e4
    DR = mybir.MatmulPerfMode.DoubleRow
    Exp = mybir.ActivationFunctionType.Exp

    nc = bacc.Bacc(
        "TRN2", target_bir_lowering=False, debug=False, num_devices=NCORES
    )

    # --- per-core DRAM I/O ---
    xq_d = nc.dram_tensor("xq", [D, ROWS], fp8, kind="ExternalInput").ap()
    xk_d = nc.dram_tensor("xk", [D, ROWS], fp8, kind="ExternalInput").ap()
    xva_d = nc.dram_tensor("xva", [D, ROWS], fp8, kind="ExternalInput").ap()
    xvb_d = nc.dram_tensor("xvb", [D, ROWS], fp8, kind="ExternalInput").ap()
    wqa_d = nc.dram_tensor("wqa", [D, CDIM], fp8, kind="ExternalInput").ap()
    wqb_d = nc.dram_tensor("wqb", [D, CDIM], fp8, kind="ExternalInput").ap()
    wka_d = nc.dram_tensor("wka", [D, CDIM], fp8, kind="ExternalInput").ap()
    wkb_d = nc.dram_tensor("wkb", [D, CDIM], fp8, kind="ExternalInput").ap()
    wva_d = nc.dram_tensor("wva", [D, CDIM], fp8, kind="ExternalInput").ap()
    wvb_d = nc.dram_tensor("wvb", [D, CDIM], fp8, kind="ExternalInput").ap()
    bq_d = nc.dram_tensor("bq", [CDIM, 1], f32, kind="ExternalInput").ap()
    bkr_d = nc.dram_tensor("bkr", [1, CDIM], bf, kind="ExternalInput").ap()
    bv_d = nc.dram_tensor("bv", [CDIM, 1], f32, kind="ExternalInput").ap()
    woT_d = nc.dram_tensor("woT", [D, D], bf, kind="ExternalInput").ap()
    bo_d = nc.dram_tensor("bo", [1, D], bf, kind="ExternalInput").ap()
    triu_d = nc.dram_tensor("triu", [128, 128], bf, kind="ExternalInput").ap()
    out_d = nc.dram_tensor("out", [RPC, D], f32, kind="ExternalOutput").ap()

    def rearr(ap):
        return ap.rearrange("(ko ki) m -> ki ko m", ki=128)

    with tile.TileContext(nc) as tc:
        with (
            tc.tile_pool(name="sb", bufs=1) as sb,
            tc.tile_pool(name="ps", bufs=1, space="PSUM") as ps,
            tc.tile_pool(name="dram", bufs=1, space="DRAM") as dram,
        ):
            # --- constants / weights ---
            wq_a = sb.tile([128, 8, CDIM], fp8, tag="w", bufs=6)
            nc.sync.dma_start(out=wq_a, in_=rearr(wqa_d))
            wq_b = sb.tile([128, 8, CDIM], fp8, tag="w", bufs=6)
            nc.sync.dma_start(out=wq_b, in_=rearr(wqb_d))
            wk_a = sb.tile([128, 8, CDIM], fp8, tag="w", bufs=6)
            nc.sync.dma_start(out=wk_a, in_=rearr(wka_d))
            wk_b = sb.tile([128, 8, CDIM], fp8, tag="w", bufs=6)
            nc.sync.dma_start(out=wk_b, in_=rearr(wkb_d))
            bq_sb = sb.tile([CDIM, 1], f32, tag="bias", bufs=3)
            nc.sync.dma_start(out=bq_sb, in_=bq_d)
            bkr_sb = sb.tile([1, CDIM], bf, tag="bkr", bufs=1)
            nc.sync.dma_start(out=bkr_sb, in_=bkr_d)
            wv_a = sb.tile([128, 8, CDIM], fp8, tag="w", bufs=6)
            wv_b = sb.tile([128, 8, CDIM], fp8, tag="w", bufs=6)
            bv_sb = sb.tile([CDIM, 1], f32, tag="bias", bufs=3)
            bo_sb = sb.tile([1, D], bf, tag="bo", bufs=1)
            triu_sb = sb.tile([128, 128], bf, tag="triu", bufs=1)
            ones_sb = sb.tile([1, 512], bf, tag="ones", bufs=1)
            nc.vector.memset(ones_sb, 1.0)

            ident_sb = sb.tile([128, 128], bf, tag="ident", bufs=1)
            make_identity(nc, ident_sb)
            # preload the exp table set during the DMA ramp so the first real
            # exp doesn't pay the ~2.7us ACT_TABLE_LOAD
            warm_sb = sb.tile([128, 128], bf, tag="warm", bufs=1)
            nc.vector.memset(warm_sb, 1.0)
            nc.scalar.activation(
                out=warm_sb[0:1, 0:1], in_=ones_sb[0:1, 0:1], func=Exp, scale=1.0
            )
            wo3 = sb.tile([128, 8, D], bf, tag="wo", bufs=1)

            # column-halved send/recv: the AllToAll runs as two contiguous
            # half-collectives so the output projection can start after the
            # first half lands
            send_d = dram.tile([2, NCORES, CDIM, RPC // 2], bf, tag="send")
            recv_d = dram.tile([2, NCORES, CDIM, RPC // 2], bf, tag="recv")

            # per-batch persistent tiles: QT fp8 (single), KT fp8 (hi,lo pair
            # on dim 1), VT bf16 (32x scaled), V3 natural layout + ones cols
            QT, KT, VT, V3 = {}, {}, {}, {}
            for b in range(B):
                QT[b] = sb.tile([128, S], fp8, tag="qt", bufs=2, name=f"QT{b}")
                KT[b] = sb.tile([128, 2, S], fp8, tag="kt", bufs=2, name=f"KT{b}")
                VT[b] = sb.tile([128, S], bf, tag="vt", bufs=2, name=f"VT{b}")
                V3[b] = sb.tile([128, 16, 130], bf, tag="v3", bufs=2, name=f"V3{b}")
                nc.vector.memset(V3[b][:, :, 64:65], 1.0)
                nc.vector.memset(V3[b][:, :, 129:130], 1.0)

            def load_x(x_d, b, pref, ih, dtype=fp8):
                # 4 DMAs of [128, 2, 1024] (two contraction k-slices each)
                co = S * b + 1024 * ih
                ts = []
                for g in range(4):
                    t = sb.tile(
                        [128, 2, 1024], dtype, tag="xt", bufs=26,
                        name=f"{pref}{b}_{g}_{ih}",
                    )
                    src_ap = x_d[g * 256 : (g + 1) * 256, co : co + 1024]
                    nc.sync.dma_start(out=t, in_=rearr(src_ap))
                    ts.append(t)
                return ts

            def dr_accum(pt, w3, xts, n, first, last):
                # one fp8-DR term: K=1024 as 4 instrs of 2 k-slices each
                for g in range(4):
                    nc.tensor.matmul(
                        pt,
                        w3[:, 2 * g : 2 * g + 2, :],
                        xts[g][:, :, (n % 2) * 512 : (n % 2 + 1) * 512],
                        start=(first and g == 0),
                        stop=(last and g == 3),
                        perf_mode=DR,
                    )

            def proj_chunk_q(xts, n):
                pt = ps.tile([128, 512], f32, tag="proj", bufs=2, name="pproj")
                dr_accum(pt, wq_a, xts, n, True, False)
                dr_accum(pt, wq_b, xts, n, False, True)
                nc.vector.tensor_scalar_add(
                    out=QT[n // 4][:, (n % 4) * 512 : (n % 4 + 1) * 512],
                    in0=pt,
                    scalar1=bq_sb,
                )

            def proj_chunk_k(xts, n):
                pt = ps.tile([128, 512], f32, tag="proj", bufs=2, name="pproj")
                # bias first (ones-row matmul) so PSUM holds K'+bias and the
                # eviction can split hi/lo with two plain DVE ops
                nc.tensor.matmul(
                    pt, bkr_sb, ones_sb,
                    start=True, stop=False,
                )
                dr_accum(pt, wk_a, xts, n, False, False)
                dr_accum(pt, wk_b, xts, n, False, True)
                cs = slice((n % 4) * 512, (n % 4 + 1) * 512)
                ka = KT[n // 4][:, 0, cs]
                nc.vector.tensor_copy(out=ka, in_=pt)
                nc.vector.tensor_sub(out=KT[n // 4][:, 1, cs], in0=pt, in1=ka)

            def proj_chunk_v(xats, xbts, n):
                pt = ps.tile([128, 512], f32, tag="proj", bufs=2, name="pproj")
                dr_accum(pt, wv_a, xats, n, True, False)
                dr_accum(pt, wv_b, xats, n, False, False)
                dr_accum(pt, wv_a, xbts, n, False, True)
                nc.vector.tensor_scalar_add(
                    out=VT[n // 4][:, (n % 4) * 512 : (n % 4 + 1) * 512],
                    in0=pt,
                    scalar1=bv_sb,
                )

            def v_tile(b, rt):
                # transpose one [128, 128] tile of VT into natural layout; per
                # head laid out [V_h | ones] in V3 (ones cols preset).
                pv = ps.tile([128, 128], bf, tag="proj", bufs=2, name="pvt")
                nc.tensor.transpose(
                    pv, VT[b][:, rt * 128 : (rt + 1) * 128], ident_sb
                )
                nc.vector.tensor_copy(out=V3[b][:, rt, 0:64], in_=pv[:, 0:64])
                nc.vector.tensor_copy(out=V3[b][:, rt, 65:129], in_=pv[:, 64:128])

            def proj_block(b, ih):
                # the 6 projection chains feeding batch b's i-half ih, as a
                # list of thunks so they can be injected into earlier loops
                xq_ts = load_x(xq_d, b, "xq", ih)
                xk_ts = load_x(xk_d, b, "xk", ih)
                xva_ts = load_x(xva_d, b, "xva", ih)
                xvb_ts = load_x(xvb_d, b, "xvb", ih)
                n0, n1 = 4 * b + 2 * ih, 4 * b + 2 * ih + 1
                return [
                    lambda: proj_chunk_q(xq_ts, n0),
                    lambda: proj_chunk_q(xq_ts, n1),
                    lambda: proj_chunk_k(xk_ts, n0),
                    lambda: proj_chunk_k(xk_ts, n1),
                    lambda: proj_chunk_v(xva_ts, xvb_ts, n0),
                    lambda: proj_chunk_v(xva_ts, xvb_ts, n1),
                ]

            def attention(b, hl, ih, fillers, vlag=0):
                # fillers: jt -> list of thunks (next-phase PE work) injected
                # so TensorE stays fed while ScalarE paces the exp stream.
                # vlag defers V-dependent work (v_tile / PV / normalize) by
                # vlag iterations so the scores+exp stream isn't blocked (PE
                # executes in program order) while the V projection's x data
                # is still in flight.
                pb = 64 * hl
                ibase = 1024 * ih
                njt = 8 * (ih + 1)
                cps = {}
                for ic in (2 * ih, 2 * ih + 1):
                    cps[ic] = ps.tile(
                        [128, 512], f32, tag="ctx", bufs=2, name=f"cps{b}{hl}{ic}"
                    )
                ex_t = {}
                for idx in range(njt + vlag):
                    for f in fillers.get(idx, ()):
                        f()
                    if idx < njt:
                        jt = idx
                        jpos = 128 * jt
                        estart = max(jpos, ibase)
                        off0 = estart - ibase
                        ex = sb.tile([128, 1024], bf, tag="ex", bufs=12, name="ex")
                        ex_t[jt] = (ex, off0)
                        sc = ps.tile([128, 1024], f32, tag="sc", bufs=2, name="sc")
                        off = off0
                        while off < 1024:
                            cw = min(512 - off % 512, 1024 - off)
                            nc.tensor.matmul(
                                sc[:, off : off + cw],
                                KT[b][pb : pb + 64, :, jpos : jpos + 128],
                                QT[b][pb : pb + 64, ibase + off : ibase + off + cw]
                                .unsqueeze(1)
                                .broadcast_to([64, 2, cw]),
                                start=True,
                                stop=True,
                                perf_mode=DR,
                            )
                            off += cw
                        nc.scalar.activation(
                            out=ex[:, off0:1024],
                            in_=sc[:, off0:1024],
                            func=Exp,
                            scale=EXPSCALE,
                        )
                        if jt // 8 == ih:
                            # diagonal block lives in this i-half: mask it
                            dg = jpos - ibase
                            nc.vector.tensor_mul(
                                ex[:, dg : dg + 128], ex[:, dg : dg + 128], triu_sb
                            )
                    vjt = idx - vlag
                    if vjt < 0 or vjt >= njt:
                        continue
                    jpos = 128 * vjt
                    ex, _ = ex_t.pop(vjt)
                    if hl == 0 and vjt // 8 == ih:
                        v_tile(b, vjt)
                    # PV: diagonal i-chunk is partial width (cols < jpos are
                    # masked and never touched)
                    for ic in (2 * ih, 2 * ih + 1):
                        if 512 * (ic + 1) <= jpos:
                            continue
                        lo = max(512 * ic, jpos)
                        nc.tensor.matmul(
                            cps[ic][0:65, lo - 512 * ic : 512],
                            V3[b][:, vjt, 65 * hl : 65 * hl + 65],
                            ex[:, lo - ibase : 512 * (ic + 1) - ibase],
                            start=(vjt == 0),
                            stop=(vjt == 4 * ic + 3),
                        )
                    if vjt % 4 == 3 and vjt // 4 in cps:
                        # chunk ic finished accumulating: normalize (PSUM row
                        # 64 holds the softmax denominator), free its slot
                        ic = vjt // 4
                        rs = sb.tile([128, 512], f32, tag="rs", bufs=3, name="rs")
                        # cross-base DVE op: read PSUM p64, write SBUF p0
                        # (partition_broadcast HW broadcasts partition 0)
                        nc.vector.reciprocal(out=rs[0:1, :], in_=cps[ic][64:65, :])
                        rb = sb.tile([64, 512], f32, tag="rb", bufs=3, name="rb")
                        nc.gpsimd.partition_broadcast(rb[0:64, :], rs[0:1, :])
                        cn = sb.tile([64, 512], bf, tag="cn", bufs=4, name="cn")
                        nc.vector.tensor_mul(cn[0:64, :], cps[ic][0:64, :], rb)
                        for h in range(2):
                            nc.sync.dma_start(
                                out=send_d[h, 4 * b + ic, pb : pb + 64, :],
                                in_=cn[0:64, h * 256 : (h + 1) * 256],
                            )

            def spread(thunks, jts):
                return {jt: [t] for jt, t in zip(jts, thunks)}

            # --- software pipeline: batch-0 prologue; attention phases run
            # (hl=0, hl=1) back-to-back per i-half so each x half-load feeds
            # two phases; later phases carry upcoming projections as fillers.
            # The first phase defers V-dependent work (vlag) so the exp
            # stream starts as soon as Q/K land, ~6us before x_v arrives ---
            xq00 = load_x(xq_d, 0, "xq", 0)
            xk00 = load_x(xk_d, 0, "xk", 0)
            proj_chunk_q(xq00, 0)
            proj_chunk_q(xq00, 1)
            proj_chunk_k(xk00, 0)
            proj_chunk_k(xk00, 1)
            # V-side weights/consts load behind the exp-critical Q/K path
            nc.sync.dma_start(out=wv_a, in_=rearr(wva_d))
            nc.sync.dma_start(out=wv_b, in_=rearr(wvb_d))
            nc.sync.dma_start(out=bv_sb, in_=bv_d)
            nc.sync.dma_start(out=triu_sb, in_=triu_d)
            nc.sync.dma_start(out=bo_sb, in_=bo_d)
            xva00 = load_x(xva_d, 0, "xva", 0)
            xvb00 = load_x(xvb_d, 0, "xvb", 0)
            v00 = lambda: proj_chunk_v(xva00, xvb00, 0)
            v01 = lambda: proj_chunk_v(xva00, xvb00, 1)
            pb01 = proj_block(0, 1)  # [Q2, Q3, K2, K3, V2, V3] of batch 0
            # V00/V01 run as drain-slot fillers inside the first phase (their
            # x is still streaming; all scores are already emitted by then)
            attention(0, 0, 0, spread((v00, v01, pb01[0]), (8, 9, 11)), vlag=8)
            attention(0, 1, 0, spread((pb01[1], pb01[2], pb01[3]), (1, 3, 5)))
            # o-proj weights: load in the DMA lull while attention runs
            nc.sync.dma_start(out=wo3, in_=rearr(woT_d))
            pb10 = proj_block(1, 0)
            attention(0, 0, 1, spread(pb01[4:] + pb10[:2], (1, 4, 8, 12)))
            pb11 = proj_block(1, 1)
            attention(0, 1, 1, spread(pb10[2:] + pb11[:2], (1, 3, 6, 9, 12, 14)))
            attention(1, 0, 0, spread(pb11[2:4], (2, 5)))
            attention(1, 1, 0, spread(pb11[4:], (1, 4)))
            attention(1, 0, 1, {})
            attention(1, 1, 1, {})

            # keep the PE HAM-warm across the collective wait: a serial
            # MM->copy->MM chain trickles ~1 matmul/us with no dependency on
            # the collective, so the output projection starts at full clock
            def trickle(n):
                for _ in range(n):
                    pwk = ps.tile([128, 128], f32, tag="sc", bufs=2, name="pwk")
                    nc.tensor.matmul(
                        pwk, ones_sb[:, 0:128], warm_sb[0:1, :], start=True, stop=True
                    )
                    nc.vector.tensor_copy(out=warm_sb, in_=pwk)

            trickle(4)

            # --- all-to-all: chunk r of my send goes to core r; run as two
            # column-half collectives so recv/oproj pipeline behind half 1 ---
            def a2a(h):
                if with_collective:
                    nc.gpsimd.collective_compute(
                        "AllToAll",
                        mybir.AluOpType.bypass,
                        replica_groups=[list(range(NCORES))],
                        ins=[send_d[h].opt()],
                        outs=[recv_d[h].opt()],
                    )
                else:
                    # timing-only stand-in (TimelineSim has no collectives)
                    nc.gpsimd.dma_start(out=recv_d[h], in_=send_d[h])

            a2a(0)
            a2a(1)
            # trickle keeps the PE p-state warm during the collective wait
            trickle(6)
            # per-it recv loads so the output projection pipelines with DMA
            rcv = sb.tile([128, 8, RPC], bf, tag="rcv", bufs=1)
            for it in range(4):
                nc.sync.dma_start(
                    out=rcv[:, :, it * 128 : (it + 1) * 128],
                    in_=recv_d[it // 2][:, :, (it % 2) * 128 : (it % 2) * 128 + 128]
                    .rearrange("p c r -> c p r"),
                )

            # --- output projection for my 512-row block ---
            for it in range(4):
                po0 = ps.tile([128, 512], f32, tag="ctx", bufs=2, name="po0")
                po1 = ps.tile([128, 512], f32, tag="ctx", bufs=2, name="po1")
                nc.tensor.matmul(po0, ones_sb[:, 0:128], bo_sb[:, 0:512], start=True, stop=False)
                nc.tensor.matmul(po1, ones_sb[:, 0:128], bo_sb[:, 512:1024], start=True, stop=False)
                for p in range(NCORES):
                    lhsT = rcv[:, p, it * 128 : (it + 1) * 128]
                    nc.tensor.matmul(
                        po0, lhsT, wo3[:, p, 0:512], start=False, stop=(p == 7)
                    )
                    nc.tensor.matmul(
                        po1, lhsT, wo3[:, p, 512:1024], start=False, stop=(p == 7)
                    )
                ob = sb.tile([128, D], f32, tag="ob", bufs=2, name="ob")
                nc.vector.tensor_copy(out=ob[:, 0:512], in_=po0)
                nc.vector.tensor_copy(out=ob[:, 512:1024], in_=po1)
                nc.sync.dma_start(out=out_d[it * 128 : (it + 1) * 128, :], in_=ob)

    nc.compile()
    return nc


def _prep_inputs(q, k, v, w_q, b_q, w_k, b_k, w_v, b_v, w_o, b_o):
    def f8(x):
        return np.ascontiguousarray(x).astype(F8)

    def bf(x):
        return np.ascontiguousarray(x).astype(BF16)

    q = np.asarray(q, np.float32).reshape(ROWS, D)
    k = np.asarray(k, np.float32).reshape(ROWS, D)
    v = np.asarray(v, np.float32).reshape(ROWS, D)
    xq = f8(q.T)
    xk = f8(k.T)
    vT = np.ascontiguousarray(v.T)
    xva = vT.astype(F8)
    xvb = f8(vT - xva.astype(np.float32))
    w_q = np.asarray(w_q, np.float32)
    w_k = np.asarray(w_k, np.float32)
    w_v = np.asarray(w_v, np.float32)
    w_o = np.asarray(w_o, np.float32)
    woT = bf(w_o.T / WS)
    bo = bf(np.asarray(b_o, np.float32).reshape(1, D))
    triu = np.triu(np.ones((128, 128), np.float32)).astype(BF16)

    def wsplit(w, hs):
        ws = WS * w[hs, :].T  # [D, CDIM]
        wa = ws.astype(F8)
        wb = (ws - wa.astype(np.float32)).astype(F8)
        return np.ascontiguousarray(wa), np.ascontiguousarray(wb)

    in_maps = []
    for c in range(NCORES):
        hs = slice(c * CDIM, (c + 1) * CDIM)
        wqa, wqb = wsplit(w_q, hs)
        wka, wkb = wsplit(w_k, hs)
        wva, wvb = wsplit(w_v, hs)
        in_maps.append(
            {
                "xq": xq,
                "xk": xk,
                "xva": xva,
                "xvb": xvb,
                "wqa": wqa, "wqb": wqb,
                "wka": wka, "wkb": wkb,
                "wva": wva, "wvb": wvb,
                "bq": np.ascontiguousarray(
                    (WS * np.asarray(b_q, np.float32)[hs]).reshape(CDIM, 1)
                ),
                "bkr": bf((WS * np.asarray(b_k, np.float32)[hs]).reshape(1, CDIM)),
                "bv": np.ascontiguousarray(
                    (WS * np.asarray(b_v, np.float32)[hs]).reshape(CDIM, 1)
                ),
                "woT": woT,
                "bo": bo,
                "triu": triu,
            }
        )
    return in_maps


def kernel(q, k, v, mask, w_q, b_q, w_k, b_k, w_v, b_v, w_o, b_o):
    global LAST_RESULTS
    if "nc" not in _CACHE:
        _CACHE["nc"] = _build_program()
    nc = _CACHE["nc"]

    from concourse.bass_utils import run_bass_kernel_spmd

    in_maps = _prep_inputs(q, k, v, w_q, b_q, w_k, b_k, w_v, b_v, w_o, b_o)
    res = run_bass_kernel_spmd(nc, in_maps, core_ids=list(range(NCORES)))
    LAST_RESULTS = res
    out = np.concatenate(
        [np.asarray(res.results[c]["out"], np.float32) for c in range(NCORES)], axis=0
    )
    return out.reshape(B, S, D)


# revision 22
# speedup vs baseline: 1.0097x; 1.0097x over previous
"""MultiHeadAttentionBlock (B=2, S=2048, D=1024, H=16, causal) on 8 trn2 cores.

Sharding: tensor-parallel over heads (2 heads / core); an on-device AllToAll
redistributes per-head context so each core computes the output projection for
its 512-row block. Host only slices / casts inputs and concatenates outputs.

fp8 DoubleRow acceleration (0.5 PE cycles/row vs 1.0 for bf16), with
error-compensated operand splits keeping rel err ~1.3e-2 (< 2e-2):
  - Q/K/V projections: weights W' = 32*W split into fp8 hi+lo (host side);
    x_q/x_k quantized to single fp8, x_v compensated (hi+lo). Q/K: 2 terms
    (Wa xa, Wb xa); K adds its bias via a ones-row matmul into PSUM so the
    eviction can split K into an fp8 (hi, lo) pair; V: 3 terms.
  - Scores: one DR matmul per j-tile: lhsT slices = (K_hi, K_lo), rhs = Q_hi
    broadcast to both slices -> K exactly compensated, 0.5 cycles/row. The
    exp scale 2^-13 absorbs the 32*32 weight scaling.
  - PV + output projection stay bf16 (softmax weights don't survive fp8);
    w_o is pre-divided by 32 host-side to undo the V scale.

Scheduling notes (the sim serializes ~625ns of HWDGE descriptor work per DMA,
and PE/DVE/Act execute in program order):
  - each x half-load is ONE [128, 8, 1024] DMA; weight hi/lo pairs are packed
    into one tensor -> the front is wire-limited, not descriptor-limited.
  - attention phases run (hl=0, hl=1) back-to-back per i-half; projection
    chunks are injected as fillers so the PE stays fed under the exp stream.
  - the first phase defers V-dependent work (vlag) so scores+exp start as
    soon as Q/K land, ~6us before x_v arrives.
  - a ScalarE-paced matmul trickle keeps the PE p-state warm at startup and
    across the collective wait (an idle gap resets the clock ramp).
  - the AllToAll runs as two column-half collectives; recv is 4 per-it tiles
    so the output projection pipelines with the recv DMA.
"""

import os
import numpy as np
import ml_dtypes

B, S, D = 2, 2048, 1024
H, DK = 16, 64
ROWS = B * S  # 4096
NCORES = 8
CDIM = 128  # context dims per core (2 heads x 64)
RPC = ROWS // NCORES  # 512 rows per core

BF16 = ml_dtypes.bfloat16
F8 = ml_dtypes.float8_e4m3
WS = 32.0  # weight prescale for fp8
EXPSCALE = 0.125 / (WS * WS)  # 2^-13

_CACHE = {}
LAST_RESULTS = None


def _build_program(with_collective=True):
    import concourse.mybir as mybir
    import concourse.tile as tile
    from concourse import bacc
    from concourse.masks import make_identity

    f32 = mybir.dt.float32
    bf = mybir.dt.bfloat16
    fp8 = mybir.dt.float8e4
    DR = mybir.MatmulPerfMode.DoubleRow
    Exp = mybir.ActivationFunctionType.Exp

    nc = bacc.Bacc(
        "TRN2", target_bir_lowering=False, debug=False, num_devices=NCORES
    )

    # --- per-core DRAM I/O ---
    xq_d = nc.dram_tensor("xq", [D, ROWS], fp8, kind="ExternalInput").ap()
    xk_d = nc.dram_tensor("xk", [D, ROWS], fp8, kind="ExternalInput").ap()
    xv_d = nc.dram_tensor("xv", [D, 2 * ROWS], fp8, kind="ExternalInput").ap()
    wq_d = nc.dram_tensor("wq", [D, 2 * CDIM], fp8, kind="ExternalInput").ap()
    wk_d = nc.dram_tensor("wk", [D, 2 * CDIM], fp8, kind="ExternalInput").ap()
    wv_d = nc.dram_tensor("wv", [D, 2 * CDIM], fp8, kind="ExternalInput").ap()
    bqv_d = nc.dram_tensor("bqv", [CDIM, 2], f32, kind="ExternalInput").ap()
    bkr_d = nc.dram_tensor("bkr", [1, CDIM], bf, kind="ExternalInput").ap()
    woT_d = nc.dram_tensor("woT", [D, D], bf, kind="ExternalInput").ap()
    bo_d = nc.dram_tensor("bo", [1, D], bf, kind="ExternalInput").ap()
    triu_d = nc.dram_tensor("triu", [128, 128], bf, kind="ExternalInput").ap()
    out_d = nc.dram_tensor("out", [RPC, D], f32, kind="ExternalOutput").ap()

    def rearr(ap):
        return ap.rearrange("(ko ki) m -> ki ko m", ki=128)

    with tile.TileContext(nc) as tc:
        with (
            tc.tile_pool(name="sb", bufs=1) as sb,
            tc.tile_pool(name="ps", bufs=1, space="PSUM") as ps,
            tc.tile_pool(name="dram", bufs=1, space="DRAM") as dram,
        ):
            # --- constants (no DMA deps) ---
            ones_sb = sb.tile([1, 512], bf, tag="ones", bufs=1)
            nc.vector.memset(ones_sb, 1.0)
            ident_sb = sb.tile([128, 128], bf, tag="ident", bufs=1)
            make_identity(nc, ident_sb)
            warm_sb = sb.tile([128, 128], bf, tag="warm", bufs=1)
            nc.vector.memset(warm_sb, 1.0)
            # preload the exp table set so the first real exp doesn't pay the
            # ~2.7us ACT_TABLE_LOAD
            nc.scalar.activation(
                out=warm_sb[0:1, 0:1], in_=ones_sb[0:1, 0:1], func=Exp, scale=1.0
            )

            # ScalarE-paced PE trickle: keeps the tensor engine's p-state ramp
            # alive through DMA-bound stretches without touching DVE
            def trickle(n):
                for _ in range(n):
                    pwk = ps.tile([128, 128], f32, tag="sc", bufs=2, name="pwk")
                    nc.tensor.matmul(
                        pwk, ones_sb[:, 0:128], warm_sb[0:1, :], start=True,
                        stop=True,
                    )
                    nc.scalar.copy(out=warm_sb, in_=pwk)

            trickle(6)

            # --- weights / biases (packed: one DMA per projection) ---
            wq3 = sb.tile([128, 8, 2 * CDIM], fp8, tag="w", bufs=3)
            nc.sync.dma_start(out=wq3, in_=rearr(wq_d))
            xq00 = sb.tile([128, 8, 1024], fp8, tag="xt", bufs=9, name="xq00")
            nc.sync.dma_start(out=xq00, in_=rearr(xq_d[:, 0:1024]))
            wk3 = sb.tile([128, 8, 2 * CDIM], fp8, tag="w", bufs=3)
            nc.sync.dma_start(out=wk3, in_=rearr(wk_d))
            xk00 = sb.tile([128, 8, 1024], fp8, tag="xt", bufs=9, name="xk00")
            nc.sync.dma_start(out=xk00, in_=rearr(xk_d[:, 0:1024]))
            bqv_sb = sb.tile([CDIM, 2], f32, tag="bias", bufs=1)
            nc.sync.dma_start(out=bqv_sb, in_=bqv_d)
            bkr_sb = sb.tile([1, CDIM], bf, tag="bkr", bufs=1)
            nc.sync.dma_start(out=bkr_sb, in_=bkr_d)
            wv3 = sb.tile([128, 8, 2 * CDIM], fp8, tag="w", bufs=3)
            nc.sync.dma_start(out=wv3, in_=rearr(wv_d))
            # xv holds xv_hi | xv_lo concatenated along rows (host layout)
            xv00 = sb.tile([128, 8, 1024], fp8, tag="xt", bufs=9, name="xva00")
            nc.sync.dma_start(out=xv00, in_=rearr(xv_d[:, 0:1024]))
            xw00 = sb.tile([128, 8, 1024], fp8, tag="xt", bufs=9, name="xvb00")
            nc.sync.dma_start(out=xw00, in_=rearr(xv_d[:, ROWS : ROWS + 1024]))
            triu_sb = sb.tile([128, 128], bf, tag="triu", bufs=1)
            nc.sync.dma_start(out=triu_sb, in_=triu_d)
            bo_sb = sb.tile([1, D], bf, tag="bo", bufs=1)
            nc.sync.dma_start(out=bo_sb, in_=bo_d)
            wo3 = sb.tile([128, 8, D], bf, tag="wo", bufs=1)

            # column-halved send/recv: the AllToAll runs as two contiguous
            # half-collectives so the output projection starts after half 1
            send_d = dram.tile([2, NCORES, CDIM, RPC // 2], bf, tag="send")
            recv_d = dram.tile([2, NCORES, CDIM, RPC // 2], bf, tag="recv")

            # per-batch persistent tiles: QT fp8 (single), KT fp8 (hi,lo pair
            # on dim 1), VT bf16 (32x scaled), V3 natural layout + ones cols
            QT, KT, VT, V3 = {}, {}, {}, {}
            for b in range(B):
                QT[b] = sb.tile([128, S], fp8, tag="qt", bufs=2, name=f"QT{b}")
                KT[b] = sb.tile([128, 2, S], fp8, tag="kt", bufs=2, name=f"KT{b}")
                VT[b] = sb.tile([128, S], bf, tag="vt", bufs=2, name=f"VT{b}")
                V3[b] = sb.tile([128, 16, 130], bf, tag="v3", bufs=2, name=f"V3{b}")
                nc.vector.memset(V3[b][:, :, 64:65], 1.0)
                nc.vector.memset(V3[b][:, :, 129:130], 1.0)

            def load_x(x_d, b, pref, ih, off=0):
                # ONE [128, 8, 1024] DMA per (tensor, batch, i-half): the sim
                # charges ~625ns of serialized descriptor time per DMA, so
                # count matters more than bytes
                co = off + S * b + 1024 * ih
                t = sb.tile(
                    [128, 8, 1024], fp8, tag="xt", bufs=9, name=f"{pref}{b}_{ih}"
                )
                nc.sync.dma_start(out=t, in_=rearr(x_d[:, co : co + 1024]))
                return t

            def dr_accum(pt, w3, wslc, xt, n, first, last):
                # one fp8-DR term: K=1024 as 4 instrs of 2 k-slices each
                for g in range(4):
                    nc.tensor.matmul(
                        pt,
                        w3[:, 2 * g : 2 * g + 2, wslc],
                        xt[:, 2 * g : 2 * g + 2, (n % 2) * 512 : (n % 2 + 1) * 512],
                        start=(first and g == 0),
                        stop=(last and g == 3),
                        perf_mode=DR,
                    )

            A, Bs = slice(0, CDIM), slice(CDIM, 2 * CDIM)

            def proj_chunk_q(xt, n):
                pt = ps.tile([128, 512], f32, tag="proj", bufs=2, name="pproj")
                dr_accum(pt, wq3, A, xt, n, True, False)
                dr_accum(pt, wq3, Bs, xt, n, False, True)
                nc.vector.tensor_scalar_add(
                    out=QT[n // 4][:, (n % 4) * 512 : (n % 4 + 1) * 512],
                    in0=pt,
                    scalar1=bqv_sb[:, 0:1],
                )

            def proj_chunk_k(xt, n):
                pt = ps.tile([128, 512], f32, tag="proj", bufs=2, name="pproj")
                # bias first (ones-row matmul) so PSUM holds K'+bias and the
                # eviction can split hi/lo with two plain DVE ops
                nc.tensor.matmul(pt, bkr_sb, ones_sb, start=True, stop=False)
                dr_accum(pt, wk3, A, xt, n, False, False)
                dr_accum(pt, wk3, Bs, xt, n, False, True)
                cs = slice((n % 4) * 512, (n % 4 + 1) * 512)
                ka = KT[n // 4][:, 0, cs]
                nc.vector.tensor_copy(out=ka, in_=pt)
                nc.vector.tensor_sub(out=KT[n // 4][:, 1, cs], in0=pt, in1=ka)

            def proj_chunk_v(xat, xbt, n):
                pt = ps.tile([128, 512], f32, tag="proj", bufs=2, name="pproj")
                dr_accum(pt, wv3, A, xat, n, True, False)
                dr_accum(pt, wv3, Bs, xat, n, False, False)
                dr_accum(pt, wv3, A, xbt, n, False, True)
                nc.vector.tensor_scalar_add(
                    out=VT[n // 4][:, (n % 4) * 512 : (n % 4 + 1) * 512],
                    in0=pt,
                    scalar1=bqv_sb[:, 1:2],
                )

            def v_tile(b, rt):
                # transpose one [128, 128] tile of VT into natural layout; per
                # head laid out [V_h | ones] in V3 (ones cols preset).
                pv = ps.tile([128, 128], bf, tag="proj", bufs=2, name="pvt")
                nc.tensor.transpose(
                    pv, VT[b][:, rt * 128 : (rt + 1) * 128], ident_sb
                )
                nc.vector.tensor_copy(out=V3[b][:, rt, 0:64], in_=pv[:, 0:64])
                nc.vector.tensor_copy(out=V3[b][:, rt, 65:129], in_=pv[:, 64:128])

            def proj_block(b, ih):
                # the 6 projection chains feeding batch b's i-half ih, as
                # thunks so they can be injected into earlier loops
                xq_t = load_x(xq_d, b, "xq", ih)
                xk_t = load_x(xk_d, b, "xk", ih)
                xva_t = load_x(xv_d, b, "xva", ih)
                xvb_t = load_x(xv_d, b, "xvb", ih, off=ROWS)
                n0, n1 = 4 * b + 2 * ih, 4 * b + 2 * ih + 1
                return [
                    lambda: proj_chunk_q(xq_t, n0),
                    lambda: proj_chunk_q(xq_t, n1),
                    lambda: proj_chunk_k(xk_t, n0),
                    lambda: proj_chunk_k(xk_t, n1),
                    lambda: proj_chunk_v(xva_t, xvb_t, n0),
                    lambda: proj_chunk_v(xva_t, xvb_t, n1),
                ]

            def attention(b, hl, ih, fillers, vlag=0):
                # fillers: idx -> thunks (next-phase PE work) injected so the
                # PE stays fed while ScalarE paces the exp stream. vlag
                # defers V-dependent work (v_tile / PV / normalize) so the
                # scores+exp stream isn't blocked (PE runs in program order)
                # while the V projection's x data is still in flight.
                pb = 64 * hl
                ibase = 1024 * ih
                njt = 8 * (ih + 1)
                cps = {}
                for ic in (2 * ih, 2 * ih + 1):
                    cps[ic] = ps.tile(
                        [128, 512], f32, tag="ctx", bufs=2, name=f"cps{b}{hl}{ic}"
                    )
                ex_t = {}
                for idx in range(njt + vlag):
                    for f in fillers.get(idx, ()):
                        f()
                    if idx < njt:
                        jt = idx
                        jpos = 128 * jt
                        estart = max(jpos, ibase)
                        off0 = estart - ibase
                        ex = sb.tile([128, 1024], bf, tag="ex", bufs=12, name="ex")
                        ex_t[jt] = ex
                        sc = ps.tile([128, 1024], f32, tag="sc", bufs=2, name="sc")
                        off = off0
                        while off < 1024:
                            cw = min(512 - off % 512, 1024 - off)
                            nc.tensor.matmul(
                                sc[:, off : off + cw],
                                KT[b][pb : pb + 64, :, jpos : jpos + 128],
                                QT[b][pb : pb + 64, ibase + off : ibase + off + cw]
                                .unsqueeze(1)
                                .broadcast_to([64, 2, cw]),
                                start=True,
                                stop=True,
                                perf_mode=DR,
                            )
                            off += cw
                        nc.scalar.activation(
                            out=ex[:, off0:1024],
                            in_=sc[:, off0:1024],
                            func=Exp,
                            scale=EXPSCALE,
                        )
                        if jt // 8 == ih:
                            # diagonal block lives in this i-half: mask it
                            dg = jpos - ibase
                            nc.vector.tensor_mul(
                                ex[:, dg : dg + 128], ex[:, dg : dg + 128], triu_sb
                            )
                    vjt = idx - vlag
                    if vjt < 0 or vjt >= njt:
                        continue
                    jpos = 128 * vjt
                    ex = ex_t.pop(vjt)
                    if hl == 0 and vjt // 8 == ih:
                        v_tile(b, vjt)
                    # PV: diagonal i-chunk is partial width (cols < jpos are
                    # masked and never touched)
                    for ic in (2 * ih, 2 * ih + 1):
                        if 512 * (ic + 1) <= jpos:
                            continue
                        lo = max(512 * ic, jpos)
                        nc.tensor.matmul(
                            cps[ic][0:65, lo - 512 * ic : 512],
                            V3[b][:, vjt, 65 * hl : 65 * hl + 65],
                            ex[:, lo - ibase : 512 * (ic + 1) - ibase],
                            start=(vjt == 0),
                            stop=(vjt == 4 * ic + 3),
                        )
                    if vjt % 4 == 3 and vjt // 4 in cps:
                        # chunk ic finished accumulating: normalize (PSUM row
                        # 64 holds the softmax denominator), free its slot
                        ic = vjt // 4
                        rs = sb.tile([128, 512], f32, tag="rs", bufs=3, name="rs")
                        # cross-base DVE op: read PSUM p64, write SBUF p0
                        # (partition_broadcast HW broadcasts partition 0)
                        nc.vector.reciprocal(out=rs[0:1, :], in_=cps[ic][64:65, :])
                        rb = sb.tile([64, 512], f32, tag="rb", bufs=3, name="rb")
                        nc.gpsimd.partition_broadcast(rb[0:64, :], rs[0:1, :])
                        cn = sb.tile([64, 512], bf, tag="cn", bufs=4, name="cn")
                        nc.vector.tensor_mul(cn[0:64, :], cps[ic][0:64, :], rb)
                        for h in range(2):
                            nc.sync.dma_start(
                                out=send_d[h, 4 * b + ic, pb : pb + 64, :],
                                in_=cn[0:64, h * 256 : (h + 1) * 256],
                            )

            def spread(thunks, idxs):
                return {i: [t] for i, t in zip(idxs, thunks)}

            # --- software pipeline ---
            proj_chunk_q(xq00, 0)
            proj_chunk_q(xq00, 1)
            proj_chunk_k(xk00, 0)
            proj_chunk_k(xk00, 1)
            v00 = lambda: proj_chunk_v(xv00, xw00, 0)
            v01 = lambda: proj_chunk_v(xv00, xw00, 1)
            pb01 = proj_block(0, 1)  # [Q2, Q3, K2, K3, V2, V3] of batch 0
            # V00/V01 run as drain-slot fillers inside the first phase (their
            # x is still streaming; all scores are already emitted by then)
            attention(0, 0, 0, spread((v00, v01, pb01[0]), (8, 9, 11)), vlag=8)
            attention(0, 1, 0, spread((pb01[1], pb01[2], pb01[3]), (1, 3, 5)))
            # o-proj weights: load in the DMA lull while attention runs
            nc.sync.dma_start(out=wo3, in_=rearr(woT_d))
            pb10 = proj_block(1, 0)
            attention(
                0, 0, 1,
                spread((pb01[4], pb10[0], pb01[5], pb10[1]), (7, 10, 11, 13)),
            )
            pb11 = proj_block(1, 1)
            attention(0, 1, 1, spread(pb10[2:] + pb11[:2], (1, 3, 6, 9, 12, 14)))
            attention(1, 0, 0, spread(pb11[2:4], (2, 5)))
            attention(1, 1, 0, spread(pb11[4:], (1, 4)))
            attention(1, 0, 1, {})
            attention(1, 1, 1, {})

            trickle(4)

            # --- all-to-all: chunk r of my send goes to core r; run as two
            # column-half collectives so recv/oproj pipeline behind half 1 ---
            def a2a(h):
                if with_collective:
                    nc.gpsimd.collective_compute(
                        "AllToAll",
                        mybir.AluOpType.bypass,
                        replica_groups=[list(range(NCORES))],
                        ins=[send_d[h].opt()],
                        outs=[recv_d[h].opt()],
                    )
                else:
                    # timing-only stand-in (TimelineSim has no collectives)
                    nc.gpsimd.dma_start(out=recv_d[h], in_=send_d[h])

            a2a(0)
            a2a(1)
            # trickle keeps the PE p-state warm during the collective wait
            trickle(8)
            # per-it recv tiles (separate tiles: one shared tile would
            # serialize recv DMA behind oproj reads via a false WAR hazard)
            rcv_t = []
            for it in range(4):
                r = sb.tile([128, 8, 128], bf, tag="rcv", bufs=4, name=f"rcv{it}")
                nc.sync.dma_start(
                    out=r,
                    in_=recv_d[it // 2][:, :, (it % 2) * 128 : (it % 2) * 128 + 128]
                    .rearrange("p c r -> c p r"),
                )
                rcv_t.append(r)

            # --- output projection for my 512-row block ---
            for it in range(4):
                po0 = ps.tile([128, 512], f32, tag="ctx", bufs=2, name="po0")
                po1 = ps.tile([128, 512], f32, tag="ctx", bufs=2, name="po1")
                nc.tensor.matmul(
                    po0, ones_sb[:, 0:128], bo_sb[:, 0:512], start=True, stop=False
                )
                nc.tensor.matmul(
                    po1, ones_sb[:, 0:128], bo_sb[:, 512:1024], start=True,
                    stop=False,
                )
                for p in range(NCORES):
                    lhsT = rcv_t[it][:, p, :]
                    nc.tensor.matmul(
                        po0, lhsT, wo3[:, p, 0:512], start=False, stop=(p == 7)
                    )
                    nc.tensor.matmul(
                        po1, lhsT, wo3[:, p, 512:1024], start=False, stop=(p == 7)
                    )
                ob = sb.tile([128, D], f32, tag="ob", bufs=2, name="ob")
                nc.vector.tensor_copy(out=ob[:, 0:512], in_=po0)
                nc.vector.tensor_copy(out=ob[:, 512:1024], in_=po1)
                nc.sync.dma_start(out=out_d[it * 128 : (it + 1) * 128, :], in_=ob)

    nc.compile()
    return nc


def _prep_inputs(q, k, v, w_q, b_q, w_k, b_k, w_v, b_v, w_o, b_o):
    def f8(x):
        return np.ascontiguousarray(x).astype(F8)

    def bf(x):
        return np.ascontiguousarray(x).astype(BF16)

    q = np.asarray(q, np.float32).reshape(ROWS, D)
    k = np.asarray(k, np.float32).reshape(ROWS, D)
    v = np.asarray(v, np.float32).reshape(ROWS, D)
    xq = f8(q.T)
    xk = f8(k.T)
    vT = np.ascontiguousarray(v.T)
    xva = vT.astype(F8)
    xvb = (vT - xva.astype(np.float32)).astype(F8)
    xv = np.ascontiguousarray(np.concatenate([xva, xvb], axis=1))
    w_q = np.asarray(w_q, np.float32)
    w_k = np.asarray(w_k, np.float32)
    w_v = np.asarray(w_v, np.float32)
    w_o = np.asarray(w_o, np.float32)
    woT = bf(w_o.T / WS)
    bo = bf(np.asarray(b_o, np.float32).reshape(1, D))
    triu = np.triu(np.ones((128, 128), np.float32)).astype(BF16)

    def wsplit(w, hs):
        ws = WS * w[hs, :].T  # [D, CDIM]
        wa = ws.astype(F8)
        wb = (ws - wa.astype(np.float32)).astype(F8)
        return np.ascontiguousarray(np.concatenate([wa, wb], axis=1))

    in_maps = []
    for c in range(NCORES):
        hs = slice(c * CDIM, (c + 1) * CDIM)
        bqv = np.stack(
            [
                WS * np.asarray(b_q, np.float32)[hs],
                WS * np.asarray(b_v, np.float32)[hs],
            ],
            axis=1,
        )
        in_maps.append(
            {
                "xq": xq,
                "xk": xk,
                "xv": xv,
                "wq": wsplit(w_q, hs),
                "wk": wsplit(w_k, hs),
                "wv": wsplit(w_v, hs),
                "bqv": np.ascontiguousarray(bqv),
                "bkr": bf((WS * np.asarray(b_k, np.float32)[hs]).reshape(1, CDIM)),
                "woT": woT,
                "bo": bo,
                "triu": triu,
            }
        )
    return in_maps


def kernel(q, k, v, mask, w_q, b_q, w_k, b_k, w_v, b_v, w_o, b_o):
    global LAST_RESULTS
    if "nc" not in _CACHE:
        _CACHE["nc"] = _build_program()
    nc = _CACHE["nc"]

    from concourse.bass_utils import run_bass_kernel_spmd

    in_maps = _prep_inputs(q, k, v, w_q, b_q, w_k, b_k, w_v, b_v, w_o, b_o)
    res = run_bass_kernel_spmd(nc, in_maps, core_ids=list(range(NCORES)))
    LAST_RESULTS = res
    out = np.concatenate(
        [np.asarray(res.results[c]["out"], np.float32) for c in range(NCORES)], axis=0
    )
    return out.reshape(B, S, D)


# revision 27
# speedup vs baseline: 1.0393x; 1.0293x over previous
"""MultiHeadAttentionBlock (B=2, S=2048, D=1024, H=16, causal) on 8 trn2 cores.

Sharding: tensor-parallel over heads (2 heads / core); an on-device AllToAll
redistributes per-head context so each core computes the output projection for
its 512-row block. Host only slices / casts inputs and concatenates outputs.

fp8 DoubleRow acceleration (0.5 PE cycles/row vs 1.0 for bf16), with
error-compensated operand splits keeping rel err ~1.3e-2 (< 2e-2):
  - Q/K/V projections: weights W' = 32*W split into fp8 hi+lo (host side);
    x_q/x_k quantized to single fp8, x_v compensated (hi+lo). Q/K: 2 terms
    (Wa xa, Wb xa); K adds its bias via a ones-row matmul into PSUM so the
    eviction can split K into an fp8 (hi, lo) pair; V: 3 terms.
  - Scores: one DR matmul per j-tile: lhsT slices = (K_hi, K_lo), rhs = Q_hi
    broadcast to both slices -> K exactly compensated, 0.5 cycles/row. The
    exp scale 2^-13 absorbs the 32*32 weight scaling.
  - PV + output projection stay bf16 (softmax weights don't survive fp8);
    w_o is pre-divided by 32 host-side to undo the V scale.

Scheduling notes (the sim serializes ~625ns of HWDGE descriptor work per DMA,
and PE/DVE/Act execute in program order):
  - each x half-load is ONE [128, 8, 1024] DMA; weight hi/lo pairs are packed
    into one tensor -> the front is wire-limited, not descriptor-limited.
  - attention phases run (hl=0, hl=1) back-to-back per i-half; projection
    chunks are injected as fillers so the PE stays fed under the exp stream.
  - the first phase defers V-dependent work (vlag) so scores+exp start as
    soon as Q/K land, ~6us before x_v arrives.
  - a ScalarE-paced matmul trickle keeps the PE p-state warm at startup and
    across the collective wait (an idle gap resets the clock ramp).
  - the AllToAll runs as two column-half collectives; recv is 4 per-it tiles
    so the output projection pipelines with the recv DMA.
"""

import os
import numpy as np
import ml_dtypes

B, S, D = 2, 2048, 1024
H, DK = 16, 64
ROWS = B * S  # 4096
NCORES = 8
CDIM = 128  # context dims per core (2 heads x 64)
RPC = ROWS // NCORES  # 512 rows per core

BF16 = ml_dtypes.bfloat16
F8 = ml_dtypes.float8_e4m3
WS = 32.0  # weight prescale for fp8
EXPSCALE = 0.125 / (WS * WS)  # 2^-13

_CACHE = {}
LAST_RESULTS = None


def _build_program(with_collective=True):
    import concourse.mybir as mybir
    import concourse.tile as tile
    from concourse import bacc
    from concourse.masks import make_identity

    f32 = mybir.dt.float32
    bf = mybir.dt.bfloat16
    fp8 = mybir.dt.float8e4
    DR = mybir.MatmulPerfMode.DoubleRow
    Exp = mybir.ActivationFunctionType.Exp

    nc = bacc.Bacc(
        "TRN2", target_bir_lowering=False, debug=False, num_devices=NCORES
    )

    # --- per-core DRAM I/O ---
    xq_d = nc.dram_tensor("xq", [D, ROWS], fp8, kind="ExternalInput").ap()
    xk_d = nc.dram_tensor("xk", [D, ROWS], fp8, kind="ExternalInput").ap()
    xv_d = nc.dram_tensor("xv", [D, 2 * ROWS], fp8, kind="ExternalInput").ap()
    wq_d = nc.dram_tensor("wq", [D, 2 * CDIM], fp8, kind="ExternalInput").ap()
    wk_d = nc.dram_tensor("wk", [D, 2 * CDIM], fp8, kind="ExternalInput").ap()
    wv_d = nc.dram_tensor("wv", [D, 2 * CDIM], fp8, kind="ExternalInput").ap()
    bqv_d = nc.dram_tensor("bqv", [CDIM, 2], f32, kind="ExternalInput").ap()
    bkr_d = nc.dram_tensor("bkr", [1, CDIM], bf, kind="ExternalInput").ap()
    woT_d = nc.dram_tensor("woT", [D, D], bf, kind="ExternalInput").ap()
    bo_d = nc.dram_tensor("bo", [1, D], bf, kind="ExternalInput").ap()
    triu_d = nc.dram_tensor("triu", [128, 128], bf, kind="ExternalInput").ap()
    out_d = nc.dram_tensor("out", [RPC, D], f32, kind="ExternalOutput").ap()

    def rearr(ap):
        return ap.rearrange("(ko ki) m -> ki ko m", ki=128)

    with tile.TileContext(nc) as tc:
        with (
            tc.tile_pool(name="sb", bufs=1) as sb,
            tc.tile_pool(name="ps", bufs=1, space="PSUM") as ps,
            tc.tile_pool(name="dram", bufs=1, space="DRAM") as dram,
        ):
            # --- constants (no DMA deps) ---
            ones_sb = sb.tile([1, 512], bf, tag="ones", bufs=1)
            nc.vector.memset(ones_sb, 1.0)
            ident_sb = sb.tile([128, 128], bf, tag="ident", bufs=1)
            make_identity(nc, ident_sb)
            warm_sb = sb.tile([128, 128], bf, tag="warm", bufs=1)
            nc.vector.memset(warm_sb, 1.0)
            # preload the exp table set so the first real exp doesn't pay the
            # ~2.7us ACT_TABLE_LOAD
            nc.scalar.activation(
                out=warm_sb[0:1, 0:1], in_=ones_sb[0:1, 0:1], func=Exp, scale=1.0
            )

            # ScalarE-paced PE trickle: keeps the tensor engine's p-state ramp
            # alive through DMA-bound stretches without touching DVE
            def trickle(n):
                for _ in range(n):
                    pwk = ps.tile([128, 128], f32, tag="sc", bufs=2, name="pwk")
                    nc.tensor.matmul(
                        pwk, ones_sb[:, 0:128], warm_sb[0:1, :], start=True,
                        stop=True,
                    )
                    nc.scalar.copy(out=warm_sb, in_=pwk)

            trickle(6)

            # --- weights / biases (packed: one DMA per projection; biases
            # queued before x so evictions never stall on them) ---
            wq3 = sb.tile([128, 8, 2 * CDIM], fp8, tag="w", bufs=3)
            nc.sync.dma_start(out=wq3, in_=rearr(wq_d))
            wk3 = sb.tile([128, 8, 2 * CDIM], fp8, tag="w", bufs=3)
            nc.sync.dma_start(out=wk3, in_=rearr(wk_d))
            bqv_sb = sb.tile([CDIM, 2], f32, tag="bias", bufs=1)
            nc.sync.dma_start(out=bqv_sb, in_=bqv_d)
            bkr_sb = sb.tile([1, CDIM], bf, tag="bkr", bufs=1)
            nc.sync.dma_start(out=bkr_sb, in_=bkr_d)
            xq00 = sb.tile([128, 8, 1024], fp8, tag="xt", bufs=11, name="xq00")
            nc.sync.dma_start(out=xq00, in_=rearr(xq_d[:, 0:1024]))
            xk00 = sb.tile([128, 8, 1024], fp8, tag="xt", bufs=11, name="xk00")
            nc.sync.dma_start(out=xk00, in_=rearr(xk_d[:, 0:1024]))
            wv3 = sb.tile([128, 8, 2 * CDIM], fp8, tag="w", bufs=3)
            nc.sync.dma_start(out=wv3, in_=rearr(wv_d))
            # xv holds xv_hi | xv_lo concatenated along rows (host layout)
            xv00 = sb.tile([128, 8, 1024], fp8, tag="xt", bufs=11, name="xva00")
            nc.sync.dma_start(out=xv00, in_=rearr(xv_d[:, 0:1024]))
            xw00 = sb.tile([128, 8, 1024], fp8, tag="xt", bufs=11, name="xvb00")
            nc.sync.dma_start(out=xw00, in_=rearr(xv_d[:, ROWS : ROWS + 1024]))
            triu_sb = sb.tile([128, 128], bf, tag="triu", bufs=1)
            nc.sync.dma_start(out=triu_sb, in_=triu_d)
            bo_sb = sb.tile([1, D], bf, tag="bo", bufs=1)
            nc.sync.dma_start(out=bo_sb, in_=bo_d)
            wo3 = sb.tile([128, 8, D], bf, tag="wo", bufs=1)

            # column-halved send/recv: the AllToAll runs as two contiguous
            # half-collectives so the output projection starts after half 1
            send_d = dram.tile([2, NCORES, CDIM, RPC // 2], bf, tag="send")
            recv_d = dram.tile([2, NCORES, CDIM, RPC // 2], bf, tag="recv")

            # per-batch persistent tiles: QT fp8 (single), KT fp8 (hi,lo pair
            # on dim 1), VT bf16 (32x scaled), V3 natural layout + ones cols
            QT, KT, VT, V3 = {}, {}, {}, {}
            for b in range(B):
                QT[b] = sb.tile([128, S], fp8, tag="qt", bufs=2, name=f"QT{b}")
                KT[b] = sb.tile([128, 2, S], fp8, tag="kt", bufs=2, name=f"KT{b}")
                VT[b] = sb.tile([128, S], bf, tag="vt", bufs=2, name=f"VT{b}")
                V3[b] = sb.tile([128, 16, 130], bf, tag="v3", bufs=2, name=f"V3{b}")
                nc.vector.memset(V3[b][:, :, 64:65], 1.0)
                nc.vector.memset(V3[b][:, :, 129:130], 1.0)

            def load_x(x_d, b, pref, ih, off=0):
                # ONE [128, 8, 1024] DMA per (tensor, batch, i-half): the sim
                # charges ~625ns of serialized descriptor time per DMA, so
                # count matters more than bytes
                co = off + S * b + 1024 * ih
                t = sb.tile(
                    [128, 8, 1024], fp8, tag="xt", bufs=11, name=f"{pref}{b}_{ih}"
                )
                nc.sync.dma_start(out=t, in_=rearr(x_d[:, co : co + 1024]))
                return t

            def dr_accum(pt, w3, wslc, xt, n, first, last):
                # one fp8-DR term: K=1024 as 4 instrs of 2 k-slices each
                for g in range(4):
                    nc.tensor.matmul(
                        pt,
                        w3[:, 2 * g : 2 * g + 2, wslc],
                        xt[:, 2 * g : 2 * g + 2, (n % 2) * 512 : (n % 2 + 1) * 512],
                        start=(first and g == 0),
                        stop=(last and g == 3),
                        perf_mode=DR,
                    )

            A, Bs = slice(0, CDIM), slice(CDIM, 2 * CDIM)

            def proj_chunk_q(xt, n):
                pt = ps.tile([128, 512], f32, tag="proj", bufs=2, name="pproj")
                dr_accum(pt, wq3, A, xt, n, True, False)
                dr_accum(pt, wq3, Bs, xt, n, False, True)
                nc.vector.tensor_scalar_add(
                    out=QT[n // 4][:, (n % 4) * 512 : (n % 4 + 1) * 512],
                    in0=pt,
                    scalar1=bqv_sb[:, 0:1],
                )

            def proj_chunk_k(xt, n):
                pt = ps.tile([128, 512], f32, tag="proj", bufs=2, name="pproj")
                # bias first (ones-row matmul) so PSUM holds K'+bias and the
                # eviction can split hi/lo with two plain DVE ops
                nc.tensor.matmul(pt, bkr_sb, ones_sb, start=True, stop=False)
                dr_accum(pt, wk3, A, xt, n, False, False)
                dr_accum(pt, wk3, Bs, xt, n, False, True)
                cs = slice((n % 4) * 512, (n % 4 + 1) * 512)
                ka = KT[n // 4][:, 0, cs]
                nc.vector.tensor_copy(out=ka, in_=pt)
                nc.vector.tensor_sub(out=KT[n // 4][:, 1, cs], in0=pt, in1=ka)

            def proj_chunk_v(xat, xbt, n):
                pt = ps.tile([128, 512], f32, tag="proj", bufs=2, name="pproj")
                dr_accum(pt, wv3, A, xat, n, True, False)
                dr_accum(pt, wv3, Bs, xat, n, False, False)
                dr_accum(pt, wv3, A, xbt, n, False, True)
                nc.vector.tensor_scalar_add(
                    out=VT[n // 4][:, (n % 4) * 512 : (n % 4 + 1) * 512],
                    in0=pt,
                    scalar1=bqv_sb[:, 1:2],
                )

            def v_tile(b, rt):
                # transpose one [128, 128] tile of VT into natural layout; per
                # head laid out [V_h | ones] in V3 (ones cols preset).
                pv = ps.tile([128, 128], bf, tag="proj", bufs=2, name="pvt")
                nc.tensor.transpose(
                    pv, VT[b][:, rt * 128 : (rt + 1) * 128], ident_sb
                )
                nc.vector.tensor_copy(out=V3[b][:, rt, 0:64], in_=pv[:, 0:64])
                nc.vector.tensor_copy(out=V3[b][:, rt, 65:129], in_=pv[:, 64:128])

            def proj_block(b, ih):
                # the 6 projection chains feeding batch b's i-half ih, as
                # thunks so they can be injected into earlier loops
                xq_t = load_x(xq_d, b, "xq", ih)
                xk_t = load_x(xk_d, b, "xk", ih)
                xva_t = load_x(xv_d, b, "xva", ih)
                xvb_t = load_x(xv_d, b, "xvb", ih, off=ROWS)
                n0, n1 = 4 * b + 2 * ih, 4 * b + 2 * ih + 1
                return [
                    lambda: proj_chunk_q(xq_t, n0),
                    lambda: proj_chunk_q(xq_t, n1),
                    lambda: proj_chunk_k(xk_t, n0),
                    lambda: proj_chunk_k(xk_t, n1),
                    lambda: proj_chunk_v(xva_t, xvb_t, n0),
                    lambda: proj_chunk_v(xva_t, xvb_t, n1),
                ]

            def attention(b, hl, ih, fillers, vlag=0):
                # fillers: idx -> thunks (next-phase PE work) injected so the
                # PE stays fed while ScalarE paces the exp stream. vlag
                # defers V-dependent work (v_tile / PV / normalize) so the
                # scores+exp stream isn't blocked (PE runs in program order)
                # while the V projection's x data is still in flight.
                pb = 64 * hl
                ibase = 1024 * ih
                njt = 8 * (ih + 1)
                cps = {}
                for ic in (2 * ih, 2 * ih + 1):
                    cps[ic] = ps.tile(
                        [128, 512], f32, tag="ctx", bufs=2, name=f"cps{b}{hl}{ic}"
                    )
                ex_t = {}
                for idx in range(njt + vlag):
                    for f in fillers.get(idx, ()):
                        f()
                    if idx < njt:
                        jt = idx
                        jpos = 128 * jt
                        estart = max(jpos, ibase)
                        off0 = estart - ibase
                        ex = sb.tile([128, 1024], bf, tag="ex", bufs=12, name="ex")
                        ex_t[jt] = ex
                        sc = ps.tile([128, 1024], f32, tag="sc", bufs=2, name="sc")
                        off = off0
                        while off < 1024:
                            cw = min(512 - off % 512, 1024 - off)
                            nc.tensor.matmul(
                                sc[:, off : off + cw],
                                KT[b][pb : pb + 64, :, jpos : jpos + 128],
                                QT[b][pb : pb + 64, ibase + off : ibase + off + cw]
                                .unsqueeze(1)
                                .broadcast_to([64, 2, cw]),
                                start=True,
                                stop=True,
                                perf_mode=DR,
                            )
                            off += cw
                        nc.scalar.activation(
                            out=ex[:, off0:1024],
                            in_=sc[:, off0:1024],
                            func=Exp,
                            scale=EXPSCALE,
                        )
                        if jt // 8 == ih:
                            # diagonal block lives in this i-half: mask it
                            dg = jpos - ibase
                            nc.vector.tensor_mul(
                                ex[:, dg : dg + 128], ex[:, dg : dg + 128], triu_sb
                            )
                    vjt = idx - vlag
                    if vjt < 0 or vjt >= njt:
                        continue
                    jpos = 128 * vjt
                    ex = ex_t.pop(vjt)
                    if hl == 0 and vjt // 8 == ih:
                        v_tile(b, vjt)
                    # PV: diagonal i-chunk is partial width (cols < jpos are
                    # masked and never touched)
                    for ic in (2 * ih, 2 * ih + 1):
                        if 512 * (ic + 1) <= jpos:
                            continue
                        lo = max(512 * ic, jpos)
                        nc.tensor.matmul(
                            cps[ic][0:65, lo - 512 * ic : 512],
                            V3[b][:, vjt, 65 * hl : 65 * hl + 65],
                            ex[:, lo - ibase : 512 * (ic + 1) - ibase],
                            start=(vjt == 0),
                            stop=(vjt == 4 * ic + 3),
                        )
                    if vjt % 4 == 3 and vjt // 4 in cps:
                        # chunk ic finished accumulating: normalize (PSUM row
                        # 64 holds the softmax denominator), free its slot
                        ic = vjt // 4
                        rs = sb.tile([128, 512], f32, tag="rs", bufs=3, name="rs")
                        # cross-base DVE op: read PSUM p64, write SBUF p0
                        # (partition_broadcast HW broadcasts partition 0)
                        nc.vector.reciprocal(out=rs[0:1, :], in_=cps[ic][64:65, :])
                        rb = sb.tile([64, 512], f32, tag="rb", bufs=3, name="rb")
                        nc.gpsimd.partition_broadcast(rb[0:64, :], rs[0:1, :])
                        cn = sb.tile([64, 512], bf, tag="cn", bufs=4, name="cn")
                        nc.vector.tensor_mul(cn[0:64, :], cps[ic][0:64, :], rb)
                        for h in range(2):
                            nc.sync.dma_start(
                                out=send_d[h, 4 * b + ic, pb : pb + 64, :],
                                in_=cn[0:64, h * 256 : (h + 1) * 256],
                            )

            def spread(thunks, idxs):
                return {i: [t] for i, t in zip(idxs, thunks)}

            # --- software pipeline ---
            proj_chunk_q(xq00, 0)
            proj_chunk_q(xq00, 1)
            proj_chunk_k(xk00, 0)
            proj_chunk_k(xk00, 1)
            v00 = lambda: proj_chunk_v(xv00, xw00, 0)
            v01 = lambda: proj_chunk_v(xv00, xw00, 1)
            pb01 = proj_block(0, 1)  # [Q2, Q3, K2, K3, V2, V3] of batch 0
            # V00/V01 run as drain-slot fillers inside the first phase (their
            # x is still streaming; all scores are already emitted by then).
            # Filler slots are placed so each thunk's x DMA has landed by the
            # time the PE reaches it (a stalled filler blocks the in-order
            # PE queue and starves the exp stream).
            attention(0, 0, 0, spread((v00, v01), (8, 9)), vlag=8)
            attention(0, 1, 0, spread(pb01[0:2], (3, 5)))
            pb10 = proj_block(1, 0)
            attention(
                0, 0, 1,
                spread(
                    (pb01[2], pb01[3], pb01[4], pb01[5], pb10[0], pb10[1]),
                    (2, 5, 8, 11, 13, 14),
                ),
            )
            pb11 = proj_block(1, 1)
            # o-proj weights: queued after all x loads (needed only at the
            # output projection; an early queue slot would delay x wires)
            nc.sync.dma_start(out=wo3, in_=rearr(woT_d))
            attention(
                0, 1, 1,
                spread(
                    (pb10[2], pb10[3], pb10[4], pb10[5], pb11[0], pb11[1]),
                    (1, 3, 8, 10, 12, 14),
                ),
            )
            attention(1, 0, 0, spread(pb11[2:4], (1, 3)))
            attention(1, 1, 0, spread(pb11[4:], (2, 4)))
            attention(1, 0, 1, {})
            attention(1, 1, 1, {})

            trickle(4)

            # --- all-to-all: chunk r of my send goes to core r; run as two
            # column-half collectives so recv/oproj pipeline behind half 1 ---
            def a2a(h):
                if with_collective:
                    nc.gpsimd.collective_compute(
                        "AllToAll",
                        mybir.AluOpType.bypass,
                        replica_groups=[list(range(NCORES))],
                        ins=[send_d[h].opt()],
                        outs=[recv_d[h].opt()],
                    )
                else:
                    # timing-only stand-in (TimelineSim has no collectives)
                    nc.gpsimd.dma_start(out=recv_d[h], in_=send_d[h])

            a2a(0)
            a2a(1)
            # trickle keeps the PE p-state warm during the collective wait
            trickle(6)
            # per-it recv tiles (separate tiles: one shared tile would
            # serialize recv DMA behind oproj reads via a false WAR hazard)
            rcv_t = []
            for it in range(4):
                r = sb.tile([128, 8, 128], bf, tag="rcv", bufs=4, name=f"rcv{it}")
                nc.sync.dma_start(
                    out=r,
                    in_=recv_d[it // 2][:, :, (it % 2) * 128 : (it % 2) * 128 + 128]
                    .rearrange("p c r -> c p r"),
                )
                rcv_t.append(r)

            # --- output projection for my 512-row block ---
            for it in range(4):
                po0 = ps.tile([128, 512], f32, tag="ctx", bufs=2, name="po0")
                po1 = ps.tile([128, 512], f32, tag="ctx", bufs=2, name="po1")
                nc.tensor.matmul(
                    po0, ones_sb[:, 0:128], bo_sb[:, 0:512], start=True, stop=False
                )
                nc.tensor.matmul(
                    po1, ones_sb[:, 0:128], bo_sb[:, 512:1024], start=True,
                    stop=False,
                )
                for p in range(NCORES):
                    lhsT = rcv_t[it][:, p, :]
                    nc.tensor.matmul(
                        po0, lhsT, wo3[:, p, 0:512], start=False, stop=(p == 7)
                    )
                    nc.tensor.matmul(
                        po1, lhsT, wo3[:, p, 512:1024], start=False, stop=(p == 7)
                    )
                ob = sb.tile([128, D], f32, tag="ob", bufs=2, name="ob")
                nc.vector.tensor_copy(out=ob[:, 0:512], in_=po0)
                nc.vector.tensor_copy(out=ob[:, 512:1024], in_=po1)
                nc.sync.dma_start(out=out_d[it * 128 : (it + 1) * 128, :], in_=ob)

    nc.compile()
    return nc


def _prep_inputs(q, k, v, w_q, b_q, w_k, b_k, w_v, b_v, w_o, b_o):
    def f8(x):
        return np.ascontiguousarray(x).astype(F8)

    def bf(x):
        return np.ascontiguousarray(x).astype(BF16)

    q = np.asarray(q, np.float32).reshape(ROWS, D)
    k = np.asarray(k, np.float32).reshape(ROWS, D)
    v = np.asarray(v, np.float32).reshape(ROWS, D)
    xq = f8(q.T)
    xk = f8(k.T)
    vT = np.ascontiguousarray(v.T)
    xva = vT.astype(F8)
    xvb = (vT - xva.astype(np.float32)).astype(F8)
    xv = np.ascontiguousarray(np.concatenate([xva, xvb], axis=1))
    w_q = np.asarray(w_q, np.float32)
    w_k = np.asarray(w_k, np.float32)
    w_v = np.asarray(w_v, np.float32)
    w_o = np.asarray(w_o, np.float32)
    woT = bf(w_o.T / WS)
    bo = bf(np.asarray(b_o, np.float32).reshape(1, D))
    triu = np.triu(np.ones((128, 128), np.float32)).astype(BF16)

    def wsplit(w, hs):
        ws = WS * w[hs, :].T  # [D, CDIM]
        wa = ws.astype(F8)
        wb = (ws - wa.astype(np.float32)).astype(F8)
        return np.ascontiguousarray(np.concatenate([wa, wb], axis=1))

    in_maps = []
    for c in range(NCORES):
        hs = slice(c * CDIM, (c + 1) * CDIM)
        bqv = np.stack(
            [
                WS * np.asarray(b_q, np.float32)[hs],
                WS * np.asarray(b_v, np.float32)[hs],
            ],
            axis=1,
        )
        in_maps.append(
            {
                "xq": xq,
                "xk": xk,
                "xv": xv,
                "wq": wsplit(w_q, hs),
                "wk": wsplit(w_k, hs),
                "wv": wsplit(w_v, hs),
                "bqv": np.ascontiguousarray(bqv),
                "bkr": bf((WS * np.asarray(b_k, np.float32)[hs]).reshape(1, CDIM)),
                "woT": woT,
                "bo": bo,
                "triu": triu,
            }
        )
    return in_maps


def kernel(q, k, v, mask, w_q, b_q, w_k, b_k, w_v, b_v, w_o, b_o):
    global LAST_RESULTS
    if "nc" not in _CACHE:
        _CACHE["nc"] = _build_program()
    nc = _CACHE["nc"]

    from concourse.bass_utils import run_bass_kernel_spmd

    in_maps = _prep_inputs(q, k, v, w_q, b_q, w_k, b_k, w_v, b_v, w_o, b_o)
    res = run_bass_kernel_spmd(nc, in_maps, core_ids=list(range(NCORES)))
    LAST_RESULTS = res
    out = np.concatenate(
        [np.asarray(res.results[c]["out"], np.float32) for c in range(NCORES)], axis=0
    )
    return out.reshape(B, S, D)
